# revision 7
# baseline (speedup 1.0000x reference)
"""Trainium2 Bass kernel for attention-score softmax (general/Luong attention).

Math: the reference computes
    proj   = einsum('sbf,hf->bsh', encoder_outputs, W) + b     # [B,S,H]
    scores = einsum('bh,bsh->bs', hidden[0], proj)[:, None, :]  # [B,1,S]
    out    = softmax(scores, axis=-1)
which algebraically reduces (scores[b,s] = (hidden[b] @ W) . enc[s,b]) to a
per-batch matvec against a precomputed v = hidden[0] @ W  [B, 2H].  The bias b
contributes hidden[b].b, constant over s, which cancels exactly in softmax
(and b is all-zeros anyway), so it is omitted.

Sharding: data-parallel over batch B=64 across 8 NeuronCores (8 batches per
core).  Each core reads its enc slice [S=2048, 8, F=1024] (64 MiB, the memory
roofline), computes v on-device (PE), the dot products on DVE via fused
tensor_tensor_reduce, and an on-device softmax over S.
"""

import numpy as np
from contextlib import ExitStack

import concourse.bass as bass
import concourse.tile as tile
import concourse.bass_isa as bass_isa
from concourse import bacc, mybir
from concourse.bass_utils import run_bass_kernel_spmd
from concourse.masks import make_identity

S, B, H = 2048, 64, 512
F = 2 * H          # encoder feature dim
NC = 8             # cores
BL = B // NC       # batches per core
P = 128            # SBUF partitions
ST = S // P        # s-tiles per core
F32 = mybir.dt.float32


def _emit_body(pools, tc: tile.TileContext, out_ap, enc_ap, hid_ap, w_ap):
    nc = tc.nc
    mult = mybir.AluOpType.mult
    add = mybir.AluOpType.add

    consts, encp, scr, psum = pools

    # ---- setup: v_bc[p, b, f] = (hid @ W)[b, f] broadcast over partitions p
    ident = consts.tile([P, P], F32)
    make_identity(nc, ident)
    h_nat = consts.tile([BL, H], F32)
    nc.sync.dma_start(h_nat, hid_ap)
    w_sb = consts.tile([P, H // P, F], F32)
    nc.sync.dma_start(w_sb, w_ap.rearrange("(c p) f -> p c f", p=P))

    # hT[:, c, :] = hid[:, c*128:(c+1)*128].T  via PE transpose
    hT = consts.tile([P, H // P, BL], F32)
    for c in range(H // P):
        ps_t = psum.tile([P, BL], F32, name="ps_t")
        nc.tensor.transpose(ps_t, h_nat[:, c * P:(c + 1) * P], ident[:BL, :BL])
        nc.scalar.copy(hT[:, c, :], ps_t)

    # v[b, f] = sum_h hid[b, h] W[h, f]
    v_sb = consts.tile([BL, F], F32)
    for n in range(F // 512):
        ps_v = psum.tile([BL, 512], F32, name="ps_v")
        for c in range(H // P):
            nc.tensor.matmul(
                ps_v, hT[:, c, :], w_sb[:, c, n * 512:(n + 1) * 512],
                start=(c == 0), stop=(c == H // P - 1),
            )
        nc.scalar.copy(v_sb[:, n * 512:(n + 1) * 512], ps_v)

    # broadcast each v row across all 128 partitions: v_bc[:,b,:] =
    # onehot[:,b,:].T @ v_sb, where onehot[k,b,m] = (k==b) (so every output
    # partition m gets row b).  PE operands must start at partition 0, which
    # rules out reading v_sb[b:b+1] directly.
    onehot = consts.tile([BL, BL, P], F32)
    nc.gpsimd.memset(onehot, 0.0)
    nc.gpsimd.affine_select(
        out=onehot, in_=onehot,
        compare_op=mybir.AluOpType.not_equal, fill=1.0, base=0,
        # val = k*1 + b*(-1) + m*0; fill 1.0 where val == 0
        pattern=[[-1, BL], [0, P]], channel_multiplier=1,
    )
    v_bc = consts.tile([P, BL, F], F32)
    for b in range(BL):
        for n in range(F // 512):
            ps_b = psum.tile([P, 512], F32, name="ps_b")
            nc.tensor.matmul(
                ps_b, onehot[:, b, :], v_sb[:, n * 512:(n + 1) * 512],
                start=True, stop=True,
            )
            nc.scalar.copy(v_bc[:, b, n * 512:(n + 1) * 512], ps_b)

    # ---- main loop: scores[p, b, t] = enc[t*128+p, b, :] . v[b, :]
    scores = consts.tile([P, BL, ST], F32)
    for t in range(ST):
        enc_t = encp.tile([P, BL, F], F32, name="enc_t")
        nc.sync.dma_start(enc_t, enc_ap[t * P:(t + 1) * P, :, :])
        for b in range(BL):
            # fused tensor_tensor_reduce is not supported by this runtime;
            # split: DVE multiply, then Act-engine per-partition sum via
            # activation accum_out (keeps DVE and Act both under the DMA
            # roofline).
            prod = scr.tile([P, F], F32, name="prod")
            nc.vector.tensor_tensor(
                out=prod, in0=enc_t[:, b, :], in1=v_bc[:, b, :], op=mult,
            )
            junk = scr.tile([P, F], F32, name="junk")
            nc.scalar.activation(
                out=junk, in_=prod,
                func=mybir.ActivationFunctionType.Identity,
                accum_out=scores[:, b, t:t + 1],
            )

    # ---- softmax over s (s = t*128 + p spans free dim t AND partitions p)
    m1 = consts.tile([P, BL], F32)
    nc.vector.tensor_reduce(m1, scores, mybir.AxisListType.X, mybir.AluOpType.max)
    mx = consts.tile([P, BL], F32)
    nc.gpsimd.partition_all_reduce(mx, m1, channels=P, reduce_op=bass_isa.ReduceOp.max)
    neg_mx = consts.tile([P, BL], F32)
    nc.scalar.mul(neg_mx, mx, -1.0)

    esc = consts.tile([P, BL, ST], F32)
    s1 = consts.tile([P, BL], F32)
    for b in range(BL):
        nc.scalar.activation(
            out=esc[:, b, :], in_=scores[:, b, :],
            func=mybir.ActivationFunctionType.Exp,
            bias=neg_mx[:, b:b + 1], scale=1.0,
            accum_out=s1[:, b:b + 1],
        )
    ssum = consts.tile([P, BL], F32)
    nc.gpsimd.partition_all_reduce(ssum, s1, channels=P, reduce_op=bass_isa.ReduceOp.add)
    rcp = consts.tile([P, BL], F32)
    nc.vector.reciprocal(rcp, ssum)

    nrm = consts.tile([P, BL, ST], F32)
    for b in range(BL):
        nc.scalar.mul(nrm[:, b, :], esc[:, b, :], rcp[:, b:b + 1])

    # transpose [p, (b, t)] -> [(b, t), p] so the output DMA has 512B runs
    ps_o = psum.tile([P, P], F32)
    nc.tensor.transpose(ps_o, nrm, ident)
    outT = consts.tile([P, P], F32)
    nc.scalar.copy(outT, ps_o)
    nc.sync.dma_start(out_ap.rearrange("b (t p) -> (b t) p", p=P), outT)


def _build(reps: int = 1):
    nc = bacc.Bacc("TRN2", target_bir_lowering=False, debug=False)
    enc = nc.dram_tensor("enc", [S, BL, F], F32, kind="ExternalInput").ap()
    hid = nc.dram_tensor("hid", [BL, H], F32, kind="ExternalInput").ap()
    w = nc.dram_tensor("w", [H, F], F32, kind="ExternalInput").ap()
    out = nc.dram_tensor("out", [BL, S], F32, kind="ExternalOutput").ap()
    with tile.TileContext(nc) as tc:
        with ExitStack() as ctx:
            pools = (
                ctx.enter_context(tc.tile_pool(name="consts", bufs=1)),
                ctx.enter_context(tc.tile_pool(name="encp", bufs=3)),
                ctx.enter_context(tc.tile_pool(name="scr", bufs=2)),
                ctx.enter_context(tc.psum_pool(name="ps", bufs=2)),
            )
            for _ in range(reps):
                _emit_body(pools, tc, out, enc, hid, w)
    nc.compile()
    return nc


_NC_CACHE: dict[int, object] = {}


def _get_nc(reps: int = 1):
    if reps not in _NC_CACHE:
        _NC_CACHE[reps] = _build(reps)
    return _NC_CACHE[reps]


def kernel(hidden, encoder_outputs, W, b, _reps: int = 1):
    hidden = np.asarray(hidden, dtype=np.float32)
    enc = np.asarray(encoder_outputs, dtype=np.float32)
    w = np.asarray(W, dtype=np.float32)

    nc = _get_nc(_reps)
    in_maps = []
    for c in range(NC):
        sl = slice(c * BL, (c + 1) * BL)
        in_maps.append({
            "enc": np.ascontiguousarray(enc[:, sl, :]),
            "hid": np.ascontiguousarray(hidden[0, sl, :]),
            "w": w,
        })
    res = run_bass_kernel_spmd(nc, in_maps, list(range(NC)))
    out = np.concatenate(
        [res.results[c]["out"].reshape(BL, 1, S) for c in range(NC)], axis=0
    )
    return out.astype(np.float32)


# revision 21
# speedup vs baseline: 7.5872x; 7.5872x over previous
"""Trainium2 Bass kernel for attention-score softmax (general/Luong attention).

Math: the reference computes
    proj   = einsum('sbf,hf->bsh', encoder_outputs, W) + b     # [B,S,H]
    scores = einsum('bh,bsh->bs', hidden[0], proj)[:, None, :]  # [B,1,S]
    out    = softmax(scores, axis=-1)
which algebraically reduces (scores[b,s] = (hidden[b] @ W) . enc[s,b]) to a
per-batch matvec against a precomputed v = hidden[0] @ W  [B, 2H].  The bias b
contributes hidden[b].b, constant over s, which cancels exactly in softmax
(and b is all-zeros anyway), so it is omitted.

Sharding: data-parallel over batch B=64 across 8 NeuronCores (8 batches per
core).  Each core reads its enc slice [S=2048, 8, F=1024] (64 MiB), computes
v on-device (PE), the dot products on DVE, and an on-device softmax over S.

This environment has a large fixed cost per *instruction* (~30-60us,
regardless of engine or operand size — measured via reps-slope probes), so
the kernel is written to minimize instruction count: 4 giant enc DMAs
(16 MiB each), one in-place DVE multiply + one DVE reduce per chunk,
softmax with whole-tile ops and stride-0 broadcast APs, transposes done
inside DMA descriptors or one PE transpose for the output layout.
"""

import numpy as np
from contextlib import ExitStack

import concourse.bass as bass
import concourse.tile as tile
import concourse.bass_isa as bass_isa
from concourse import bacc, mybir
from concourse.bass_utils import run_bass_kernel_spmd
from concourse.masks import make_identity

S, B, H = 2048, 64, 512
F = 2 * H          # encoder feature dim
NC = 8             # cores
BL = B // NC       # batches per core
P = 128            # SBUF partitions
CH = 4             # enc chunks per core
SJ = S // (CH * P)  # 4  s-subtiles per chunk
ST = S // P        # 16 s-tiles total
F32 = mybir.dt.float32


def _emit_body(pools, tc, out_ap, enc_ap, hid_ap, w_ap, v_dram):
    nc = tc.nc
    consts, encp, psum = pools
    mult = mybir.AluOpType.mult
    add = mybir.AluOpType.add
    sub = mybir.AluOpType.subtract

    # ---- v = hid @ W on PE, then broadcast across partitions via DRAM bounce
    # hT[p, b, c] = hid[b, c*128+p]  (transpose done by the DMA descriptors)
    hT = consts.tile([P, BL, H // P], F32, name="hT")
    nc.sync.dma_start(hT, hid_ap.rearrange("b (c p) -> p b c", p=P))
    w_sb = consts.tile([P, H // P, F], F32, name="w_sb")
    nc.sync.dma_start(w_sb, w_ap.rearrange("(c p) f -> p c f", p=P))

    ps_v = psum.tile([BL, F], F32, name="ps_v")
    for n in range(F // 512):
        for c in range(H // P):
            nc.tensor.matmul(
                ps_v[:, n * 512:(n + 1) * 512],
                hT[:, :, c], w_sb[:, c, n * 512:(n + 1) * 512],
                start=(c == 0), stop=(c == H // P - 1),
            )
    v_sb = consts.tile([BL, F], F32, name="v_sb")
    nc.scalar.copy(v_sb, ps_v)
    nc.sync.dma_start(v_dram, v_sb)
    v_bc = consts.tile([P, BL, F], F32, name="v_bc")
    v_dram_bcast = bass.AP(
        tensor=v_dram.tensor, offset=v_dram.offset,
        ap=[[0, P]] + list(v_dram.ap),
    )
    nc.sync.dma_start(v_bc, v_dram_bcast)

    # ---- scores[p, t, b] = enc[t*128+p, b, :] . v[b, :]
    scores = consts.tile([P, ST, BL], F32, name="scores")
    v_bc4 = v_bc.unsqueeze(1).broadcast_to([P, SJ, BL, F])
    for c in range(CH):
        enc_t = encp.tile([P, SJ, BL, F], F32, name="enc_t")
        nc.sync.dma_start(
            enc_t,
            enc_ap[c * SJ * P:(c + 1) * SJ * P].rearrange(
                "(j p) b f -> p j b f", p=P),
        )
        nc.vector.tensor_tensor(out=enc_t, in0=enc_t, in1=v_bc4, op=mult)
        nc.vector.tensor_reduce(
            scores[:, c * SJ:(c + 1) * SJ, :], enc_t,
            mybir.AxisListType.X, add,
        )

    # ---- softmax over s  (s = t*128 + p spans free dim t AND partitions p)
    scores_bt = scores.rearrange("p t b -> p b t")
    m1 = consts.tile([P, BL], F32, name="m1")
    nc.vector.tensor_reduce(m1, scores_bt, mybir.AxisListType.X,
                            mybir.AluOpType.max)
    mx = consts.tile([P, BL], F32, name="mx")
    nc.gpsimd.partition_all_reduce(mx, m1, channels=P,
                                   reduce_op=bass_isa.ReduceOp.max)
    mx_t = mx.unsqueeze(1).broadcast_to([P, ST, BL])
    nc.vector.tensor_tensor(out=scores, in0=scores, in1=mx_t, op=sub)
    nc.scalar.activation(out=scores, in_=scores,
                         func=mybir.ActivationFunctionType.Exp)
    s1 = consts.tile([P, BL], F32, name="s1")
    nc.vector.tensor_reduce(s1, scores_bt, mybir.AxisListType.X, add)
    ssum = consts.tile([P, BL], F32, name="ssum")
    nc.gpsimd.partition_all_reduce(ssum, s1, channels=P,
                                   reduce_op=bass_isa.ReduceOp.add)
    rcp = consts.tile([P, BL], F32, name="rcp")
    nc.vector.reciprocal(rcp, ssum)
    rcp_t = rcp.unsqueeze(1).broadcast_to([P, ST, BL])
    nc.vector.tensor_tensor(out=scores, in0=scores, in1=rcp_t, op=mult)

    # out[b, t*128+p] = scores[p, t, b].  A single transposing DMA would need
    # 4 AP dims (>3 limit); a PE transpose puts (t,b) on partitions so the
    # final DMA has contiguous 512B runs.
    ident = consts.tile([P, P], F32, name="ident")
    make_identity(nc, ident)
    ps_o = psum.tile([P, P], F32, name="ps_o")
    nc.tensor.transpose(ps_o, scores, ident)
    outT = consts.tile([P, P], F32, name="outT")
    nc.scalar.copy(outT, ps_o)
    nc.sync.dma_start(out_ap.rearrange("b (t p) -> t b p", p=P), outT)


def _build(reps: int = 1):
    nc = bacc.Bacc("TRN2", target_bir_lowering=False, debug=False)
    enc = nc.dram_tensor("enc", [S, BL, F], F32, kind="ExternalInput").ap()
    hid = nc.dram_tensor("hid", [BL, H], F32, kind="ExternalInput").ap()
    w = nc.dram_tensor("w", [H, F], F32, kind="ExternalInput").ap()
    out = nc.dram_tensor("out", [BL, S], F32, kind="ExternalOutput").ap()
    v_dram = nc.dram_tensor("v_scratch", [BL, F], F32).ap()
    with tile.TileContext(nc) as tc:
        with ExitStack() as ctx:
            pools = (
                ctx.enter_context(tc.tile_pool(name="consts", bufs=1)),
                ctx.enter_context(tc.tile_pool(name="encp", bufs=1)),
                ctx.enter_context(tc.psum_pool(name="ps", bufs=1)),
            )
            for _ in range(reps):
                _emit_body(pools, tc, out, enc, hid, w, v_dram)
    nc.compile()
    return nc


_NC_CACHE: dict[int, object] = {}


def _get_nc(reps: int = 1):
    if reps not in _NC_CACHE:
        _NC_CACHE[reps] = _build(reps)
    return _NC_CACHE[reps]


def kernel(hidden, encoder_outputs, W, b, _reps: int = 1):
    hidden = np.asarray(hidden, dtype=np.float32)
    enc = np.asarray(encoder_outputs, dtype=np.float32)
    w = np.asarray(W, dtype=np.float32)

    nc = _get_nc(_reps)
    in_maps = []
    for c in range(NC):
        sl = slice(c * BL, (c + 1) * BL)
        in_maps.append({
            "enc": np.ascontiguousarray(enc[:, sl, :]),
            "hid": np.ascontiguousarray(hidden[0, sl, :]),
            "w": w,
        })
    res = run_bass_kernel_spmd(nc, in_maps, list(range(NC)))
    out = np.concatenate(
        [res.results[c]["out"].reshape(BL, 1, S) for c in range(NC)], axis=0
    )
    return out.astype(np.float32)
